# revision 1
# baseline (speedup 1.0000x reference)
"""Trainium2 Bass kernel for the spike-decoder GNN message-passing module.

Math (per batch b, output time tau in [0, T-2], variable v):
  out[b,tau,v] = bias[v]
               + sum_{i,k} w[v,i,k] * x[b,i,tau+k-(K-2)]          (static conv)
               + sum_{e: recv[e]=v} sum_k dw[e,b,tau,k] * x[b,send[e],tau+k-(K-2)]
with w = conv_weight masked at w[i,i,K-1] = 0, x = spikes[...,0] transposed to
[b, nvar, t], and out-of-range x treated as zero.

Sharding: 8 cores = (b in 0..3) x (time half h in 0..1). Each core computes a
1024-wide tau window ([0,1024) or [1023,2047) — one overlapping column keeps
shapes uniform for SPMD). dyn_weights is the only big tensor (268 MB); its
[E, 1024, K] slice per core is the memory-bound stream.

On-core algorithm (all fp32):
  - xg[e,:] = x[send[e],:] gathered via one-hot matmul on PE (exact: x is 0/1)
  - products P[e,(tau,k)] = dw_tile * sliding-window(xg) on DVE (one big
    tensor_tensor per e-tile with an overlapping stride-1 AP for the window)
  - k-reduction + recv-scatter + transpose folded into PE: for each k, a
    matmul with stationary one-hot recv matrix and moving operand = strided
    columns P[:, tau*K+k], accumulating into PSUM[v, tau]
  - static conv: 16 matmuls with stationary wT_k and shifted xpad slices
  - bias: rank-1 matmul (bias x ones)
All terms accumulate into one PSUM bank [v, 512], copied out by ScalarE.
Output is [v, tau] per core; host transposes while assembling the result.
"""

import numpy as np

B, T, NVAR, K, E = 4, 2048, 128, 16, 512
TAU = T - 1            # 2047
L = 1024               # per-core tau window
NC_COUNT = 8
W_XPAD = L + K         # 1040 (1039 used; padded even for f32r matmul ISA)
ETILES = E // 128      # 4
CHUNK = 512            # tau chunk per PSUM bank
NCHUNK = L // CHUNK    # 2

_PROGRAM = None


def _build_program():
    import concourse.bass as bass
    import concourse.bacc as bacc
    import concourse.mybir as mybir
    import concourse.tile as tile

    f32 = mybir.dt.float32
    # float32r: same fp32 bytes, but the PE streams 1 row/cycle (vs 4 for
    # strict fp32 which needs 2 half-rate passes) when the moving dim >= 256.
    f32r = mybir.dt.float32r
    bf16 = mybir.dt.bfloat16
    # Bacc (not plain Bass): its compile pipeline runs generate_event_semaphores,
    # which splits multi-semaphore waits — a raw fp32 Matmult supports only one
    # sync-wait slot and walrus rejects more ("Too many sync wait commands").
    nc = bacc.Bacc()

    xpad_d = nc.declare_dram_parameter("xpad", [NVAR, W_XPAD], f32r, isOutput=False)
    dw_d = nc.declare_dram_parameter("dw", [NCHUNK * E, CHUNK * K], f32, isOutput=False)
    ssend_d = nc.declare_dram_parameter("ssend", [NVAR, E], f32r, isOutput=False)
    wt_d = nc.declare_dram_parameter("wt", [NVAR, K * NVAR], f32r, isOutput=False)
    recv_d = nc.declare_dram_parameter("recvT", [128, ETILES * NVAR], bf16, isOutput=False)
    bo_d = nc.declare_dram_parameter("bias_ones", [1, NVAR + CHUNK], f32r, isOutput=False)
    y_d = nc.declare_dram_parameter("yT", [NVAR, L], f32, isOutput=True)

    with tile.TileContext(nc) as tc:
        with (
            tc.tile_pool(name="consts", bufs=1) as consts,
            tc.tile_pool(name="xgp", bufs=1) as xgp,
            tc.tile_pool(name="gpsum", bufs=2, space=bass.MemorySpace.PSUM) as gpsum,
            tc.tile_pool(name="dwp", bufs=3) as dwp,
            tc.tile_pool(name="prodp", bufs=3) as prodp,
            tc.tile_pool(name="opsum", bufs=2, space=bass.MemorySpace.PSUM) as opsum,
            tc.tile_pool(name="resp", bufs=2) as resp,
        ):
            NT = NCHUNK * ETILES  # 8 dw tiles
            HK = CHUNK * K // 2   # half-tile product columns (4096)
            HC = CHUNK // 2       # tau columns per half (256)

            # SP/HWDGE issue order = completion order (per-engine FIFO):
            # gather inputs first (small), then the dw stream owns the queue.
            # Tiles 0 and 7 are split into half-DMAs (16KB packets, slightly
            # slower) so the first multiply starts ~5us earlier and the tail
            # half overlaps its matmuls; middle tiles stay whole for peak
            # 32KB-packet bandwidth.
            xpad = consts.tile([NVAR, W_XPAD], f32r)
            nc.sync.dma_start(xpad[:], xpad_d[:])
            ssend = consts.tile([NVAR, E], f32r)
            nc.sync.dma_start(ssend[:], ssend_d[:])

            def dw_dma(dwt, ti, halves):
                h2, et = divmod(ti, ETILES)
                r0 = h2 * E + et * 128
                if halves:
                    for half in range(2):
                        nc.sync.dma_start(
                            dwt[:, half * HK:(half + 1) * HK],
                            dw_d[r0:r0 + 128, half * HK:(half + 1) * HK],
                        )
                else:
                    nc.sync.dma_start(dwt[:], dw_d[r0:r0 + 128, :])

            dwt_tiles = []
            for ti in range(NT):
                dwt = dwp.tile([128, CHUNK * K], f32, name="dwt", tag="dwt")
                dwt_tiles.append(dwt)
            dw_dma(dwt_tiles[0], 0, halves=True)
            # remaining small constants slot in behind the first dw tile
            wt = consts.tile([NVAR, K * NVAR], f32r)
            nc.sync.dma_start(wt[:], wt_d[:])
            recvT = consts.tile([128, ETILES * NVAR], bf16)
            nc.sync.dma_start(recvT[:], recv_d[:])
            bias_ones = consts.tile([1, NVAR + CHUNK], f32r)
            nc.sync.dma_start(bias_ones[:], bo_d[:])
            for ti in range(1, NT):
                dw_dma(dwt_tiles[ti], ti, halves=(ti == NT - 1))

            # Gather sender rows: xg[et][p, j] = xpad[send[et*128+p], j]
            xg = []
            for et in range(ETILES):
                xgt = xgp.tile([128, W_XPAD], f32, name=f"xg{et}", tag=f"xg{et}")
                for j0 in range(0, W_XPAD, CHUNK):
                    jw = min(CHUNK, W_XPAD - j0)
                    gps = gpsum.tile([128, CHUNK], f32, name="gps", tag="gps")
                    nc.tensor.matmul(
                        gps[:, :jw],
                        ssend[:, et * 128:(et + 1) * 128],
                        xpad[:, j0:j0 + jw],
                        start=True, stop=True,
                    )
                    nc.scalar.copy(xgt[:, j0:j0 + jw], gps[:, :jw])
                xg.append(xgt)

            ops_tiles = []
            for h2 in range(NCHUNK):
                o = opsum.tile([128, CHUNK], f32, name=f"ops{h2}", tag=f"ops{h2}")
                ops_tiles.append(o)

            def static_mm(h2, k, start=False):
                t0 = h2 * CHUNK
                nc.tensor.matmul(
                    ops_tiles[h2][:],
                    wt[:, k * NVAR:(k + 1) * NVAR],
                    xpad[:, t0 + k:t0 + k + CHUNK],
                    start=start, stop=False,
                )

            def bias_mm(h2):
                nc.tensor.matmul(
                    ops_tiles[h2][:],
                    bias_ones[:1, 0:NVAR],
                    bias_ones[:1, NVAR:NVAR + CHUNK],
                    start=False, stop=False,
                )

            # chunk-0 static conv + bias up front (PE warmup while dw streams)
            for k in range(K):
                static_mm(0, k, start=(k == 0))
            bias_mm(0)

            # chunk-1 static matmuls fill PE gaps across the first 7 groups
            fill = [("s", k) for k in range(K)] + [("b", None)]
            fills_per_group = [3, 3, 3, 2, 2, 2, 2, 0]

            KH = K // 2
            for ti in range(NT):
                h2, et = divmod(ti, ETILES)
                t0 = h2 * CHUNK
                dwt = dwt_tiles[ti]
                pt = prodp.tile([128, CHUNK * K], bf16, name="pt", tag="pt")
                drow = dwt.tensor.shape[-1]
                prow = pt.tensor.shape[-1]
                xrow = xg[et].tensor.shape[-1]
                # dw arrives k-major: dwt[e, k*CHUNK + tau]. Products keep that
                # layout, so every AP below is stride-1 in its innermost dim
                # (strided PE moving operands cost ~3-6 cycles/column, and
                # strided bf16 DVE writes hit sub-word read-modify-write).
                # Each tile is processed as two k-halves: the 8 matmuls of
                # half a run while DVE multiplies half b.
                for half in range(2):
                    k0 = half * KH
                    in0 = bass.AP(dwt.tensor, k0 * CHUNK,
                                  [[drow, 128], [CHUNK, KH], [1, CHUNK]])
                    # sliding window: in1[p, k, tau] = xg[p, t0 + tau + k]
                    in1 = bass.AP(xg[et].tensor, t0 + k0,
                                  [[xrow, 128], [1, KH], [1, CHUNK]])
                    out3 = bass.AP(pt.tensor, k0 * CHUNK,
                                   [[prow, 128], [CHUNK, KH], [1, CHUNK]])
                    nc.vector.tensor_mul(out3, in0, in1)
                    # k-reduction + recv scatter on PE (bf16, contiguous rhs):
                    # psum[v, tau] += sum_e recvT[e, v] * P[e, k*CHUNK + tau]
                    for k in range(k0, k0 + KH):
                        rhs = bass.AP(pt.tensor, k * CHUNK,
                                      [[prow, 128], [1, CHUNK]])
                        nc.tensor.matmul(
                            ops_tiles[h2][:],
                            recvT[:, et * NVAR:(et + 1) * NVAR],
                            rhs,
                            start=False,
                            stop=(et == ETILES - 1 and k == K - 1),
                        )
                for _ in range(fills_per_group[ti]):
                    kind, k = fill.pop(0)
                    if kind == "s":
                        static_mm(1, k, start=(k == 0))
                    else:
                        bias_mm(1)
                if et == ETILES - 1:
                    res = resp.tile([128, CHUNK], f32, name="res", tag="res")
                    nc.scalar.copy(res[:], ops_tiles[h2][:])
                    nc.gpsimd.dma_start(y_d[:, t0:t0 + CHUNK], res[:])

    nc.compile()
    return nc


def _get_program():
    global _PROGRAM
    if _PROGRAM is None:
        _PROGRAM = _build_program()
    return _PROGRAM


def _host_prep(spikes, conv_weight, conv_bias, dyn_weights, edge_send, edge_recv):
    spikes = np.asarray(spikes, dtype=np.float32)
    conv_weight = np.asarray(conv_weight, dtype=np.float32)
    conv_bias = np.asarray(conv_bias, dtype=np.float32)
    dyn_weights = np.asarray(dyn_weights, dtype=np.float32)
    edge_send = np.asarray(edge_send, dtype=np.int64)
    edge_recv = np.asarray(edge_recv, dtype=np.int64)

    x = np.ascontiguousarray(spikes[..., 0].transpose(0, 2, 1))  # [B, NVAR, T]

    ssend = np.zeros((NVAR, E), np.float32)
    ssend[edge_send, np.arange(E)] = 1.0

    import ml_dtypes
    recvT = np.zeros((128, ETILES * NVAR), ml_dtypes.bfloat16)
    for et in range(ETILES):
        rr = edge_recv[et * 128:(et + 1) * 128]
        recvT[np.arange(128), et * NVAR + rr] = 1.0

    w = conv_weight.copy()
    w[np.arange(NVAR), np.arange(NVAR), K - 1] = 0.0
    wt = np.ascontiguousarray(w.transpose(1, 2, 0)).reshape(NVAR, K * NVAR)

    bias_ones = np.concatenate(
        [conv_bias, np.ones(CHUNK, np.float32)]
    ).reshape(1, NVAR + CHUNK).astype(np.float32)

    in_maps = []
    for core in range(NC_COUNT):
        b, h = divmod(core, 2)
        tau0 = 0 if h == 0 else TAU - L  # 0 or 1023
        xpad = np.zeros((NVAR, W_XPAD), np.float32)
        lo = tau0 - (K - 2)  # first x column needed
        src_lo = max(lo, 0)
        xpad[:, src_lo - lo:W_XPAD - 1] = x[b, :, src_lo:tau0 + L + 1]
        a = dyn_weights[:, b, tau0:tau0 + L, :]          # [E, L, K]
        a = a.reshape(E, NCHUNK, CHUNK, K)               # [E, h2, tau, k]
        a = a.transpose(1, 0, 3, 2)                      # [h2, E, k, tau]
        dw = np.ascontiguousarray(a).reshape(NCHUNK * E, CHUNK * K)
        in_maps.append({
            "xpad": xpad,
            "dw": dw,
            "ssend": ssend,
            "wt": wt,
            "recvT": recvT,
            "bias_ones": bias_ones,
        })
    return in_maps


def _assemble(results):
    out = np.empty((B, TAU, NVAR, 1), np.float32)
    for core in range(NC_COUNT):
        b, h = divmod(core, 2)
        yT = results[core]["yT"]  # [NVAR, L]
        if h == 0:
            out[b, 0:L, :, 0] = yT.T
        else:
            out[b, L:TAU, :, 0] = yT[:, 1:L].T
    return out


def run_on_hw(in_maps, trace=False, **kwargs):
    from concourse.bass_utils import run_bass_kernel_spmd

    nc = _get_program()
    return run_bass_kernel_spmd(
        nc, in_maps, core_ids=list(range(NC_COUNT)), trace=trace, **kwargs
    )


def kernel(spikes, conv_weight, conv_bias, dyn_weights, edge_send, edge_recv):
    in_maps = _host_prep(
        spikes, conv_weight, conv_bias, dyn_weights, edge_send, edge_recv
    )
    res = run_on_hw(in_maps)
    return _assemble(res.results)



# revision 2
# speedup vs baseline: 1.8769x; 1.8769x over previous
"""Trainium2 Bass kernel for the spike-decoder GNN message-passing module.

Math (per batch b, output time tau in [0, T-2], variable v):
  out[b,tau,v] = bias[v]
               + sum_{i,k} w[v,i,k] * x[b,i,tau+k-(K-2)]          (static conv)
               + sum_{e: recv[e]=v} sum_k dw[e,b,tau,k] * x[b,send[e],tau+k-(K-2)]
with w = conv_weight masked at w[i,i,K-1] = 0, x = spikes[...,0] transposed to
[b, nvar, t], and out-of-range x treated as zero.

Sharding: 8 cores = (b in 0..3) x (time half h in 0..1). Each core computes a
1024-wide tau window ([0,1024) or [1023,2047) — one overlapping column keeps
shapes uniform for SPMD). dyn_weights is the only big tensor; it streams as
bf16 (the 2e-2 tolerance leaves ~10x headroom: measured 1.7e-3) which halves
the memory-bound DMA to ~17 MB/core.

On-core algorithm (bf16 operands, fp32 PSUM accumulation — exact except for
the host-side bf16 rounding of dw/w/bias, since x is 0/1):
  - xg[e,:] = x[send[e],:] gathered via one-hot matmul on PE; two SBUF copies
    (xg_even, and xg_odd shifted by one column) so every DVE window AP starts
    4B-aligned.
  - products P[e,(k,tau)] = dw_tile * sliding-window(xg) on DVE. dw arrives
    parity-major ([even ks | odd ks]); each tile takes TWO tensor_tensor ops
    (even-k reading xg_even, odd-k reading xg_odd) whose operands are all
    2-byte, stride-1, 4B-aligned -> DVE 2x_1p mode (2 elem/cycle/lane).
  - k-reduction + recv-scatter + transpose folded into PE: per k, a bf16
    matmul with stationary one-hot recv matrix and moving operand = product
    chunk P[:, kC..kC+C], accumulating into PSUM[v, tau].
  - static conv: 16 bf16 matmuls with stationary wT_k and shifted xpad slices
  - bias: rank-1 matmul (bias x ones)
All terms accumulate into one PSUM bank [v, 512], copied out by ScalarE.
Output is [v, tau] per core; host transposes while assembling the result.
"""

import numpy as np

B, T, NVAR, K, E = 4, 2048, 128, 16, 512
TAU = T - 1            # 2047
L = 1024               # per-core tau window
NC_COUNT = 8
W_XPAD = L + K         # 1040 (1039 used; even so bf16 tiles stay 4B-aligned)
ETILES = E // 128      # 4
CHUNK = 512            # tau chunk per PSUM bank
NCHUNK = L // CHUNK    # 2
KH = K // 2            # 8 ks per parity
HK = CHUNK * KH        # 4096 product columns per parity half

_PROGRAM = None


def _build_program():
    import concourse.bass as bass
    import concourse.bacc as bacc
    import concourse.mybir as mybir
    import concourse.tile as tile

    f32 = mybir.dt.float32
    bf16 = mybir.dt.bfloat16
    # Bacc (not plain Bass): its compile pipeline runs generate_event_semaphores,
    # which splits multi-semaphore waits — a raw Matmult supports only one
    # sync-wait slot and walrus rejects more ("Too many sync wait commands").
    nc = bacc.Bacc()

    xpad_d = nc.declare_dram_parameter("xpad", [NVAR, W_XPAD], bf16, isOutput=False)
    dw_d = nc.declare_dram_parameter("dw", [NCHUNK * E, CHUNK * K], bf16, isOutput=False)
    ssend_d = nc.declare_dram_parameter("ssend", [NVAR, E], bf16, isOutput=False)
    wt_d = nc.declare_dram_parameter("wt", [NVAR, K * NVAR], bf16, isOutput=False)
    recv_d = nc.declare_dram_parameter("recvT", [128, ETILES * NVAR], bf16, isOutput=False)
    bo_d = nc.declare_dram_parameter("bias_ones", [1, NVAR + CHUNK], bf16, isOutput=False)
    y_d = nc.declare_dram_parameter("yT", [NVAR, L], f32, isOutput=True)

    with tile.TileContext(nc) as tc:
        with (
            tc.tile_pool(name="consts", bufs=1) as consts,
            tc.tile_pool(name="xgp", bufs=1) as xgp,
            tc.tile_pool(name="gpsum", bufs=2, space=bass.MemorySpace.PSUM) as gpsum,
            tc.tile_pool(name="dwp", bufs=4) as dwp,
            tc.tile_pool(name="prodp", bufs=4) as prodp,
            tc.tile_pool(name="opsum", bufs=2, space=bass.MemorySpace.PSUM) as opsum,
            tc.tile_pool(name="resp", bufs=2) as resp,
        ):
            NT = NCHUNK * ETILES  # 8 dw tiles

            # SP/HWDGE issue order = completion order (per-engine FIFO):
            # gather inputs first (small), then the dw stream owns the queue.
            # Tiles 0 and 7 are split into parity-half DMAs so the first
            # multiply starts earlier and the tail half overlaps its matmuls;
            # middle tiles stay whole for peak packet size.
            xpad = consts.tile([NVAR, W_XPAD], bf16)
            nc.sync.dma_start(xpad[:], xpad_d[:])
            ssend = consts.tile([NVAR, E], bf16)
            nc.sync.dma_start(ssend[:], ssend_d[:])

            def dw_dma(dwt, ti, halves):
                h2, et = divmod(ti, ETILES)
                r0 = h2 * E + et * 128
                if halves:
                    for half in range(2):
                        nc.sync.dma_start(
                            dwt[:, half * HK:(half + 1) * HK],
                            dw_d[r0:r0 + 128, half * HK:(half + 1) * HK],
                        )
                else:
                    nc.sync.dma_start(dwt[:], dw_d[r0:r0 + 128, :])

            dwt_tiles = []
            for ti in range(NT):
                dwt = dwp.tile([128, CHUNK * K], bf16, name="dwt", tag="dwt")
                dwt_tiles.append(dwt)
            dw_dma(dwt_tiles[0], 0, halves=True)
            # remaining small constants slot in behind the first dw tile
            wt = consts.tile([NVAR, K * NVAR], bf16)
            nc.sync.dma_start(wt[:], wt_d[:])
            recvT = consts.tile([128, ETILES * NVAR], bf16)
            nc.sync.dma_start(recvT[:], recv_d[:])
            bias_ones = consts.tile([1, NVAR + CHUNK], bf16)
            nc.sync.dma_start(bias_ones[:], bo_d[:])
            for ti in range(1, NT):
                dw_dma(dwt_tiles[ti], ti, halves=(ti == NT - 1))

            # Gather sender rows: xg_even[et][p, j] = xpad[send[et*128+p], j],
            # xg_odd = the same shifted left one column (odd-k window APs then
            # start at even element offsets -> keeps DVE 2x_1p).
            xg_e, xg_o = [], []
            for et in range(ETILES):
                xge = xgp.tile([128, W_XPAD], bf16, name=f"xge{et}", tag=f"xge{et}")
                xgo = xgp.tile([128, W_XPAD], bf16, name=f"xgo{et}", tag=f"xgo{et}")
                for j0 in range(0, W_XPAD, CHUNK):
                    jw = min(CHUNK, W_XPAD - j0)
                    gps = gpsum.tile([128, CHUNK], f32, name="gps", tag="gps")
                    nc.tensor.matmul(
                        gps[:, :jw],
                        ssend[:, et * 128:(et + 1) * 128],
                        xpad[:, j0:j0 + jw],
                        start=True, stop=True,
                    )
                    nc.scalar.copy(xge[:, j0:j0 + jw], gps[:, :jw])
                    s0 = 1 if j0 == 0 else 0
                    nc.scalar.copy(xgo[:, j0 - 1 + s0:j0 + jw - 1], gps[:, s0:jw])
                xg_e.append(xge)
                xg_o.append(xgo)

            ops_tiles = []
            for h2 in range(NCHUNK):
                o = opsum.tile([128, CHUNK], f32, name=f"ops{h2}", tag=f"ops{h2}")
                ops_tiles.append(o)

            def static_mm(h2, k, start=False):
                t0 = h2 * CHUNK
                nc.tensor.matmul(
                    ops_tiles[h2][:],
                    wt[:, k * NVAR:(k + 1) * NVAR],
                    xpad[:, t0 + k:t0 + k + CHUNK],
                    start=start, stop=False,
                )

            def bias_mm(h2):
                nc.tensor.matmul(
                    ops_tiles[h2][:],
                    bias_ones[:1, 0:NVAR],
                    bias_ones[:1, NVAR:NVAR + CHUNK],
                    start=False, stop=False,
                )

            # chunk-0 static conv + bias up front (PE warmup while dw streams)
            for k in range(K):
                static_mm(0, k, start=(k == 0))
            bias_mm(0)

            # chunk-1 static matmuls fill PE gaps across the first 7 groups
            fill = [("s", k) for k in range(K)] + [("b", None)]
            fills_per_group = [3, 3, 3, 2, 2, 2, 2, 0]

            for ti in range(NT):
                h2, et = divmod(ti, ETILES)
                t0 = h2 * CHUNK
                dwt = dwt_tiles[ti]
                drow = dwt.tensor.shape[-1]
                xrow = xg_e[et].tensor.shape[-1]
                # dw arrives parity-major: dwt[e, par*HK + m*CHUNK + tau] holds
                # dw[e, k=2m+par, tau]. Each parity half is one DVE
                # tensor_tensor (all operands bf16, stride-1, 4B-aligned ->
                # 2x_1p) followed by its 8 PE reduce matmuls while the other
                # parity multiplies.
                for par, xg in ((0, xg_e[et]), (1, xg_o[et])):
                    pt = prodp.tile([128, HK], bf16, name="pt", tag="pt")
                    prow = pt.tensor.shape[-1]
                    in0 = bass.AP(dwt.tensor, par * HK,
                                  [[drow, 128], [CHUNK, KH], [1, CHUNK]])
                    # window: in1[p, m, tau] = xg[p, t0 + 2m + tau]
                    in1 = bass.AP(xg.tensor, t0,
                                  [[xrow, 128], [2, KH], [1, CHUNK]])
                    out3 = bass.AP(pt.tensor, 0,
                                   [[prow, 128], [CHUNK, KH], [1, CHUNK]])
                    nc.vector.tensor_mul(out3, in0, in1)
                    # k-reduction + recv scatter on PE (bf16, contiguous rhs):
                    # psum[v, tau] += sum_e recvT[e, v] * P[e, m*CHUNK + tau]
                    for m in range(KH):
                        rhs = bass.AP(pt.tensor, m * CHUNK,
                                      [[prow, 128], [1, CHUNK]])
                        nc.tensor.matmul(
                            ops_tiles[h2][:],
                            recvT[:, et * NVAR:(et + 1) * NVAR],
                            rhs,
                            start=False,
                            stop=(et == ETILES - 1 and par == 1 and m == KH - 1),
                        )
                for _ in range(fills_per_group[ti]):
                    kind, k = fill.pop(0)
                    if kind == "s":
                        static_mm(1, k, start=(k == 0))
                    else:
                        bias_mm(1)
                if et == ETILES - 1:
                    res = resp.tile([128, CHUNK], f32, name="res", tag="res")
                    nc.scalar.copy(res[:], ops_tiles[h2][:])
                    nc.gpsimd.dma_start(y_d[:, t0:t0 + CHUNK], res[:])

    nc.compile()
    return nc


def _get_program():
    global _PROGRAM
    if _PROGRAM is None:
        _PROGRAM = _build_program()
    return _PROGRAM


# k order inside a parity-major dw row: evens then odds
_KORDER = list(range(0, K, 2)) + list(range(1, K, 2))


def _host_prep(spikes, conv_weight, conv_bias, dyn_weights, edge_send, edge_recv):
    import ml_dtypes
    bf = ml_dtypes.bfloat16

    spikes = np.asarray(spikes, dtype=np.float32)
    conv_weight = np.asarray(conv_weight, dtype=np.float32)
    conv_bias = np.asarray(conv_bias, dtype=np.float32)
    dyn_weights = np.asarray(dyn_weights, dtype=np.float32)
    edge_send = np.asarray(edge_send, dtype=np.int64)
    edge_recv = np.asarray(edge_recv, dtype=np.int64)

    x = np.ascontiguousarray(spikes[..., 0].transpose(0, 2, 1))  # [B, NVAR, T]
    dynb = dyn_weights.astype(bf)  # one bulk fp32->bf16 pass

    ssend = np.zeros((NVAR, E), bf)
    ssend[edge_send, np.arange(E)] = 1.0

    recvT = np.zeros((128, ETILES * NVAR), bf)
    for et in range(ETILES):
        rr = edge_recv[et * 128:(et + 1) * 128]
        recvT[np.arange(128), et * NVAR + rr] = 1.0

    w = conv_weight.copy()
    w[np.arange(NVAR), np.arange(NVAR), K - 1] = 0.0
    wt = np.ascontiguousarray(w.transpose(1, 2, 0)).reshape(NVAR, K * NVAR).astype(bf)

    bias_ones = np.concatenate(
        [conv_bias, np.ones(CHUNK, np.float32)]
    ).reshape(1, NVAR + CHUNK).astype(bf)

    in_maps = []
    for core in range(NC_COUNT):
        b, h = divmod(core, 2)
        tau0 = 0 if h == 0 else TAU - L  # 0 or 1023
        xpad = np.zeros((NVAR, W_XPAD), bf)
        lo = tau0 - (K - 2)  # first x column needed
        src_lo = max(lo, 0)
        xpad[:, src_lo - lo:W_XPAD - 1] = x[b, :, src_lo:tau0 + L + 1]
        a = dynb[:, b, tau0:tau0 + L, :]                 # [E, L, K]
        a = a.reshape(E, NCHUNK, CHUNK, K)               # [E, h2, tau, k]
        a = a.transpose(1, 0, 3, 2)[:, :, _KORDER, :]    # [h2, E, kpar, tau]
        dw = np.ascontiguousarray(a).reshape(NCHUNK * E, CHUNK * K)
        in_maps.append({
            "xpad": xpad,
            "dw": dw,
            "ssend": ssend,
            "wt": wt,
            "recvT": recvT,
            "bias_ones": bias_ones,
        })
    return in_maps


def _assemble(results):
    out = np.empty((B, TAU, NVAR, 1), np.float32)
    for core in range(NC_COUNT):
        b, h = divmod(core, 2)
        yT = results[core]["yT"]  # [NVAR, L]
        if h == 0:
            out[b, 0:L, :, 0] = yT.T
        else:
            out[b, L:TAU, :, 0] = yT[:, 1:L].T
    return out


def run_on_hw(in_maps, trace=False, **kwargs):
    from concourse.bass_utils import run_bass_kernel_spmd

    nc = _get_program()
    return run_bass_kernel_spmd(
        nc, in_maps, core_ids=list(range(NC_COUNT)), trace=trace, **kwargs
    )


def kernel(spikes, conv_weight, conv_bias, dyn_weights, edge_send, edge_recv):
    in_maps = _host_prep(
        spikes, conv_weight, conv_bias, dyn_weights, edge_send, edge_recv
    )
    res = run_on_hw(in_maps)
    return _assemble(res.results)
